# revision 42
# baseline (speedup 1.0000x reference)
"""Trainium2 Bass kernel for nn_Cheb_35888746725726 (ChebConv K=3 GNN, N=50000,
E=800000, F=H=96, lambda_max=2 -> diag term is 0).

Strategy (8 NeuronCores, node/graph-parallel). The wall-clock of a call is
dominated by host->device transfer over the axon tunnel (~60-77MB/s, one
serialized pipe, ~35ms fixed per device_put, single host CPU), so inputs are
packed into TWO uint16 blobs per core (~1.72MB/core total, 13.76MB global)
and everything derivable is rebuilt on device:
 - x ships once (bf16, sharded node-major, put FIRST so its wire time
   overlaps the edge packing); an AllGather rebuilds the full node table.
 - The per-edge-tile weighted one-hot scatter matrices (128 edges x 64 dst
   slots) are built on device from compact slot/weight data via DVE
   is_equal+mult against an iota row; slots ship as uint8 pairs packed in
   uint16 and are unpacked with bitwise ops.
 - Dense weights/biases ship sharded 1/8 per core and are AllGathered.
 - Feature-major x comes from on-device PE transposes.
 - Gather indices ship as uint16 and are cast-copied to int32 on device.
 - The output is AllGathered on device so the host fetches a single shard,
   with copy_to_host_async armed at dispatch to hide the D2H sync.
Device pipeline per prop: indirect-DMA gather of source rows (bf16) from the
HBM node table -> scatter via one-hot matmuls accumulating in PSUM. Chebyshev
recurrence folded into host-modified dense weights:
  out = Tx0 @ (W0-W2) + Tx1 @ W1 + (L@Tx1) @ (2*W2), so Tx2 is never formed.
AllGather (8 cores) rebuilds the full node table between dependent props.
The PJRT executable is compiled once and cached; per-call work is vectorized
numpy preprocessing overlapped with the async device_put of the blobs.
"""
import numpy as np
import ml_dtypes

import concourse.bass as bass
import concourse.bacc as bacc
import concourse.mybir as mybir
import concourse.tile as tile
from concourse.masks import make_identity

# ---- problem constants (hardcoded per the harness contract) ----
N = 50000
E = 800000
F = 96
K = 3
C = 8                    # cores
NP_PAD = 50176           # 8 * 6272
SHARD = NP_PAD // C      # 6272
NTW = 64                 # node-tile width
NT = SHARD // NTW        # 98 node tiles / core
TE = 8                   # edge tiles (of 128 edges) per node tile
P = 128
NCHUNK = 512             # dense matmul node-chunk
N_TILES = NP_PAD // NTW  # 784 node tiles globally

BF = ml_dtypes.bfloat16

# two input blobs per core (uint16 units): the x blob ships first so its
# wire time overlaps the edge packing; the edge blob follows.
XLEN = SHARD * F                     # 602112  : x_own bf16 [SHARD, F]
E_SLOTP = 0                          # uint16 [P, NT*TE/2]: uint8 slot PAIRS
E_W = E_SLOTP + P * NT * TE // 2     # bf16 [P, NT*TE] edge weight
E_SRC = E_W + P * NT * TE            # uint16 [P, NT*TE] src node id
E_W8 = E_SRC + P * NT * TE           # this core's 1/8 slice of the weights
# weights are sharded across cores and AllGathered on device. Full layout
# (u16 units, within the gathered buffer):
W_WM = 0                             # bf16 [6F, F] folded dense weights
W_WLIN = W_WM + 6 * F * F            # bf16 [F, 2]
W_B = W_WLIN + F * 2                 # f32 [F, 2] biases (b1, b2 cols)
W_BLIN = W_B + 2 * F * 2             # f32 [2]
WLEN = W_BLIN + 4                    # 55876
WSH = 6992                           # per-core slice (8*WSH = 55936 >= WLEN)
ELEN = E_W8 + WSH

import os
DBG_NO_AG = bool(int(os.environ.get("KDBG_NO_AG", "0")))

_RT = None               # cached runtime: nc + jitted executable + metadata


# --------------------------------------------------------------------------
# host-side preprocessing (fully vectorized)
# --------------------------------------------------------------------------
def _norm_and_perm(src, dst, w):
    """Chebyshev edge norm + degree-balanced node permutation (snake)."""
    deg = np.bincount(src, weights=w.astype(np.float64), minlength=N)
    deg = deg.astype(np.float32)
    dis = np.where(deg > 0, 1.0 / np.sqrt(np.maximum(deg, 1e-30)), 0.0)
    dis = dis.astype(np.float32)
    norm_w = (-dis[src] * w * dis[dst]).astype(np.float32)

    # snake assignment of degree-sorted nodes -> balanced per-tile edge load
    indeg = np.bincount(dst, minlength=N).astype(np.int32)
    order = np.argsort(-indeg, kind="stable")
    r = np.arange(N, dtype=np.int32)
    blk = r // N_TILES
    pos = r % N_TILES
    t_r = np.where(blk % 2 == 0, pos, N_TILES - 1 - pos)
    tile_assign = np.empty(N, np.int32)
    tile_assign[order] = t_r
    slot_assign = np.empty(N, np.int32)
    slot_assign[order] = blk

    new_id = (tile_assign * NTW + slot_assign).astype(np.int32)
    return norm_w, new_id, (tile_assign, slot_assign, indeg)


class _TileOverflow(Exception):
    pass


def _repair(tile_assign, slot_assign, dst, indeg):
    """Slow path: rebalance tiles whose edge load exceeds TE*P (never taken
    for the reference input distribution; pack_edges detects the overflow
    for free from its sort)."""
    cap = TE * P
    tl = np.bincount(tile_assign[dst], minlength=N_TILES)
    for _ in range(2000):
        if tl.max() <= cap:
            break
        t_over = int(tl.argmax())
        t_under = int(tl.argmin())
        no = np.where(tile_assign == t_over)[0]
        nu = np.where(tile_assign == t_under)[0]
        a = no[np.argmax(indeg[no])]
        b = nu[np.argmin(indeg[nu])]
        tile_assign[a], tile_assign[b] = t_under, t_over
        slot_assign[a], slot_assign[b] = slot_assign[b], slot_assign[a]
        d = indeg[a] - indeg[b]
        tl[t_over] -= d
        tl[t_under] += d
    assert tl.max() <= cap, f"tile overflow after repair: {tl.max()}"
    return (tile_assign * NTW + slot_assign).astype(np.int32)


def _pack_edges(src_n, dst_n, norm_w, blob):
    """Fill per-core slot/weight/src [P, NT*TE] regions of the blob.

    One global radix sort by destination tile; edges of a tile fill its
    TE*P scatter slots in sorted order (the slot one-hot makes intra-tile
    order irrelevant). All scatters run on native uint8/uint16/int32 for
    speed on the single host CPU. Slots ship as uint8 pairs packed into
    uint16 (unpacked on device with bitwise ops).
    """
    slot8 = (dst_n & 63).astype(np.uint8)
    es16 = src_n.astype(np.uint16)
    tof16 = (dst_n >> 6).astype(np.int16)          # global tile id < 784
    o = np.argsort(tof16, kind="stable")           # radix sort
    tof = tof16[o].astype(np.int32)
    slot_s = slot8[o]
    es_s = es16[o]
    wbf = norm_w[o].astype(BF).view(np.uint16)
    starts = np.searchsorted(tof, np.arange(N_TILES, dtype=np.int32))
    if np.diff(np.append(starts, E)).max() > TE * P:
        raise _TileOverflow
    rank = np.arange(E, dtype=np.int32) - starts[tof].astype(np.int32)
    tid = np.arange(N_TILES, dtype=np.int32)
    base784 = ((tid // NT) * P) * (NT * TE) + (tid % NT) * TE
    lin = base784[tof] + (rank & (P - 1)) * (NT * TE) + (rank >> 7)

    npc = P * NT * TE
    sl = np.zeros(C * npc, np.uint8)
    sl[lin] = slot_s
    st = np.zeros((2, C * npc), np.uint16)
    st[0, lin] = wbf
    st[1, lin] = es_s

    blob[:, E_SLOTP:E_W] = sl.view(np.uint16).reshape(C, npc // 2)
    blob[:, E_W:E_SRC] = st[0].reshape(C, npc)
    blob[:, E_SRC:E_W8] = st[1].reshape(C, npc)


def _fold_weights(W1, b1, W2, b2, Wlin, blin, blob):
    wm = np.concatenate([
        W1[0] - W1[2], W1[1], 2.0 * W1[2],
        W2[0] - W2[2], W2[1], 2.0 * W2[2],
    ], axis=0).astype(BF)                                     # [6F, F]
    wall = np.zeros(C * WSH, np.uint16)
    wall[W_WM:W_WLIN] = wm.reshape(-1).view(np.uint16)
    wall[W_WLIN:W_B] = Wlin.astype(BF).reshape(-1).view(np.uint16)
    biases = np.stack([b1, b2], axis=1).astype(np.float32)    # [F, 2]
    wall[W_B:W_BLIN] = biases.reshape(-1).view(np.uint16)
    wall[W_BLIN:WLEN] = blin.astype(np.float32).reshape(-1).view(np.uint16)
    blob[:, E_W8:ELEN] = wall.reshape(C, WSH)


# --------------------------------------------------------------------------
# bass kernel builder
# --------------------------------------------------------------------------
def _build_kernel():
    dt = mybir.dt
    nc = bacc.Bacc("TRN2", target_bir_lowering=False, debug=False, num_devices=C)

    bx_d = nc.dram_tensor("bx", [XLEN], dt.uint16, kind="ExternalInput")
    be_d = nc.dram_tensor("be", [ELEN], dt.uint16, kind="ExternalInput")
    # full output is AllGathered on device so the host fetches ONE shard
    out_d = nc.dram_tensor("out", [2 * C, SHARD], dt.float32, kind="ExternalOutput")

    bx_bf = bx_d.bitcast(dt.bfloat16)
    be_bf = be_d.bitcast(dt.bfloat16)

    rg = [list(range(C))]

    with tile.TileContext(nc) as tc:
        with (
            tc.tile_pool(name="res", bufs=1) as res,          # resident sbuf
            tc.tile_pool(name="mpool", bufs=4) as mpool,      # gather dests
            tc.tile_pool(name="spool", bufs=2) as spool,      # small evac tiles
            tc.tile_pool(name="pscat", bufs=4, space="PSUM") as pscat,
            tc.tile_pool(name="ptr", bufs=2, space="PSUM") as ptr,
            tc.tile_pool(name="pdense", bufs=2, space="PSUM") as pdense,
            tc.tile_pool(name="dram", bufs=1, space="DRAM") as dram,
        ):
            # ---------- resident loads (from the packed blobs) ----------
            # weights: each core ships 1/8; AllGather rebuilds the full set
            wb = dram.tile([WSH], dt.uint16, name="wbounce")
            wfull = dram.tile([C * WSH], dt.uint16,
                              addr_space=("Local" if DBG_NO_AG else "Shared"),
                              name="wfull")
            nc.sync.dma_start(out=wb[:], in_=be_d[E_W8:ELEN])
            if DBG_NO_AG:
                for r_ in range(C):
                    nc.sync.dma_start(out=wfull[r_ * WSH:(r_ + 1) * WSH], in_=wb[:])
            else:
                nc.gpsimd.collective_compute(
                    "AllGather", mybir.AluOpType.bypass, replica_groups=rg,
                    ins=[wb[:]], outs=[wfull.opt()])

            # slot pairs -> f32 slot_sb via bitwise unpack (tensor_scalar
            # is_equal requires float32 scalar operands)
            slotp_sb = res.tile([P, NT * TE // 2], dt.uint16)
            nc.sync.dma_start(out=slotp_sb[:],
                              in_=be_d[E_SLOTP:E_W].rearrange("(p c) -> p c", p=P))
            lo_u = res.tile([P, NT * TE // 2], dt.uint16)
            hi_u = res.tile([P, NT * TE // 2], dt.uint16)
            nc.vector.tensor_scalar(out=lo_u[:], in0=slotp_sb[:], scalar1=0x00FF,
                                    scalar2=None, op0=mybir.AluOpType.bitwise_and)
            nc.vector.tensor_scalar(out=hi_u[:], in0=slotp_sb[:], scalar1=0xFF00,
                                    scalar2=None, op0=mybir.AluOpType.bitwise_and)
            slot_sb = res.tile([P, NT * TE], dt.float32)
            slot_ev = slot_sb[:].rearrange("p (c two) -> p c two", two=2)
            nc.vector.tensor_copy(
                out=slot_ev[:, :, 0:1].rearrange("p c o -> p (c o)"), in_=lo_u[:])
            hif = res.tile([P, NT * TE // 2], dt.float32)
            nc.vector.tensor_copy(out=hif[:], in_=hi_u[:])
            nc.vector.tensor_scalar(
                out=slot_ev[:, :, 1:2].rearrange("p c o -> p (c o)"),
                in0=hif[:], scalar1=1.0 / 256.0, scalar2=None,
                op0=mybir.AluOpType.mult)

            we_sb = res.tile([P, NT * TE], dt.float32)
            nc.gpsimd.dma_start(out=we_sb[:],
                                in_=be_bf[E_W:E_SRC].rearrange("(p c) -> p c", p=P))
            srcu_sb = res.tile([P, NT * TE], dt.uint16)
            nc.sync.dma_start(out=srcu_sb[:],
                              in_=be_d[E_SRC:E_W8].rearrange("(p c) -> p c", p=P))
            src_sb = res.tile([P, NT * TE], dt.int32)
            nc.vector.tensor_copy(out=src_sb[:], in_=srcu_sb[:])

            w_sb = res.tile([F, 6 * F], dt.bfloat16)   # 6 lhsT mats side by side
            for i in range(6):
                nc.sync.dma_start(
                    out=w_sb[:, i * F:(i + 1) * F],
                    in_=wfull[W_WM + i * F * F:W_WM + (i + 1) * F * F]
                        .bitcast(dt.bfloat16).rearrange("(a b) -> a b", b=F))
            wlin_sb = res.tile([F, 2], dt.bfloat16)
            nc.sync.dma_start(out=wlin_sb[:],
                              in_=wfull[W_WLIN:W_B].bitcast(dt.bfloat16)
                                  .rearrange("(f b) -> f b", b=2))
            bias_sb = res.tile([F, 2], dt.float32)
            nc.sync.dma_start(out=bias_sb[:],
                              in_=wfull[W_B:W_BLIN].bitcast(dt.float32)
                                  .rearrange("(f b) -> f b", b=2))
            blin_sb = res.tile([2, 1], dt.float32)
            nc.sync.dma_start(out=blin_sb[:],
                              in_=wfull[W_BLIN:WLEN].bitcast(dt.float32)
                                  .rearrange("(p o) -> p o", o=1))
            ident = res.tile([P, P], dt.bfloat16)
            make_identity(nc, ident[:])

            # one-hot scatter matrices built on device: oh[p, e*64+s] =
            # (s == slot[p,e]) * w[p,e]
            iota_i = res.tile([P, NTW], dt.int32)
            nc.gpsimd.iota(iota_i[:], pattern=[[1, NTW]], base=0,
                           channel_multiplier=0)
            iota_bf = res.tile([P, NTW], dt.bfloat16)
            nc.vector.tensor_copy(out=iota_bf[:], in_=iota_i[:])
            oh_sb = res.tile([P, NT * TE * NTW], dt.bfloat16)
            for e in range(NT * TE):
                nc.vector.tensor_scalar(
                    out=oh_sb[:, e * NTW:(e + 1) * NTW],
                    in0=iota_bf[:],
                    scalar1=slot_sb[:, e:e + 1],
                    scalar2=we_sb[:, e:e + 1],
                    op0=mybir.AluOpType.is_equal,
                    op1=mybir.AluOpType.mult,
                )

            # feature-major activation buffers (bf16)
            fm = {
                "tx0": res.tile([F, SHARD], dt.bfloat16, name="fm_tx0"),
                "t1": res.tile([F, SHARD], dt.bfloat16, name="fm_t1"),
                "s2": res.tile([F, SHARD], dt.bfloat16, name="fm_s2"),
                "h": res.tile([F, SHARD], dt.bfloat16, name="fm_h"),
            }

            # node-major staging for table writes / transposes
            s_nm = res.tile([P, (NT // 2) * F], dt.bfloat16)

            # internal DRAM
            bounce = [dram.tile([SHARD, F], dt.bfloat16, name=f"bounce{i}") for i in range(3)]
            bounce_x = dram.tile([SHARD, F], dt.bfloat16, name="bounce_x")
            fo = dram.tile([2, SHARD], dt.float32, name="fo")
            ag_o = dram.tile([2 * C, SHARD], dt.float32,
                             addr_space=("Local" if DBG_NO_AG else "Shared"),
                             name="ag_o")
            addr_space = "Local" if DBG_NO_AG else "Shared"
            ag = [dram.tile([NP_PAD, F], dt.bfloat16,
                            addr_space=addr_space, name=f"ag{i}")
                  for i in range(3)]
            ag_x = dram.tile([NP_PAD, F], dt.bfloat16,
                             addr_space=addr_space, name="ag_x")

            # ---------- helpers ----------
            def prop(table_ap, tag, probe=False):
                """one propagation: gather+scatter; results land in s_nm (node-major)."""
                with nc.named_scope(f"prop_{tag}"):
                    pr = None
                    if probe:
                        # tiny gpsimd-issued DMA touching the table: executes the
                        # collective-completion wait so the 1-wait-limited
                        # dynamic gathers below don't need it
                        pr = spool.tile([1, 2], dt.bfloat16, tag="pr")
                        nc.gpsimd.dma_start(out=pr[:], in_=table_ap.tensor[0:1, 0:2])
                    for nt in range(NT):
                        m_t = mpool.tile([P, TE * F], dt.bfloat16, tag="m")
                        # absorber: one strided gpsimd write touching each edge
                        # tile's corner carries the slot's WAR/WAW waits (and the
                        # table-probe dep for the first tile) so each 1-wait-
                        # limited dynamic gather below needs at most one wait.
                        # HW note: indirect DMA honors only ONE offset column
                        # per call, hence one gather per 128-edge tile.
                        if pr is not None and nt == 0:
                            nc.gpsimd.tensor_copy(out=m_t[0:1, 0:1], in_=pr[0:1, 0:1])
                        corner = m_t[:].rearrange("p (t f) -> p t f", f=F)[0:1, :, 0:1]
                        nc.gpsimd.memset(corner, 0)
                        for t in range(TE):
                            nc.gpsimd.indirect_dma_start(
                                out=m_t[:, t * F:(t + 1) * F],
                                out_offset=None,
                                in_=table_ap,
                                in_offset=bass.IndirectOffsetOnAxis(
                                    ap=src_sb[:, nt * TE + t:nt * TE + t + 1], axis=0),
                            )
                        ps = pscat.tile([NTW, F], dt.float32, space="PSUM", tag="ps")
                        for t in range(TE):
                            nc.tensor.matmul(
                                out=ps[:],
                                lhsT=oh_sb[:, (nt * TE + t) * NTW:(nt * TE + t + 1) * NTW],
                                rhs=m_t[:, t * F:(t + 1) * F],
                                start=(t == 0),
                                stop=(t == TE - 1),
                            )
                        j, b = nt // 2, nt % 2
                        nc.vector.tensor_copy(
                            out=s_nm[b * NTW:(b + 1) * NTW, j * F:(j + 1) * F],
                            in_=ps[:])

            def allgather(in_ap, out_tile, tag):
                with nc.named_scope(f"ag_{tag}"):
                    if DBG_NO_AG:
                        for r in range(C):
                            nc.sync.dma_start(
                                out=out_tile[r * SHARD:(r + 1) * SHARD, :],
                                in_=in_ap)
                    else:
                        nc.gpsimd.collective_compute(
                            "AllGather",
                            mybir.AluOpType.bypass,
                            replica_groups=rg,
                            ins=[in_ap],
                            outs=[out_tile.opt()],
                        )

            def table_write_and_ag(idx):
                """write s_nm -> bounce[idx] (node-major [SHARD, F]) and allgather."""
                bo = bounce[idx]
                view = bo[:].rearrange("(j p) f -> p j f", p=P)
                nc.sync.dma_start(out=view, in_=s_nm[:].rearrange("p (j f) -> p j f", f=F))
                allgather(bo[:], ag[idx], str(idx))

            def snm_to_fm(dest, tag):
                """transpose node-major s_nm into feature-major dest tile."""
                with nc.named_scope(f"fm_{tag}"):
                    for j in range(NT // 2):
                        pt = ptr.tile([F, P], dt.bfloat16, space="PSUM", tag="pt")
                        nc.tensor.transpose(out=pt[:], in_=s_nm[:, j * F:(j + 1) * F],
                                            identity=ident[:])
                        nc.vector.tensor_copy(out=dest[:, j * P:(j + 1) * P], in_=pt[:])

            def fm_to_snm(src_t, tag):
                """transpose feature-major tile back into s_nm node-major staging."""
                with nc.named_scope(f"nm_{tag}"):
                    for j in range(NT // 2):
                        pt = ptr.tile([P, F], dt.bfloat16, space="PSUM", tag="pt")
                        nc.tensor.transpose(out=pt[:], in_=src_t[:, j * P:(j + 1) * P],
                                            identity=ident[:F, :F])
                        nc.vector.tensor_copy(out=s_nm[:, j * F:(j + 1) * F], in_=pt[:])

            def dense(layer, tx0_t, t1_t, s2_t, h_t):
                """h = relu(tx0@W0' + t1@W1 + s2@W2') feature-major, bf16 out."""
                with nc.named_scope(f"dense_{layer}"):
                    wof = layer * 3 * F
                    nchunks = (SHARD + NCHUNK - 1) // NCHUNK
                    for ci in range(nchunks):
                        c0 = ci * NCHUNK
                        c1 = min(SHARD, c0 + NCHUNK)
                        pd = pdense.tile([F, NCHUNK], dt.float32, space="PSUM", tag="pd")
                        for ki, rhs_t in enumerate((tx0_t, t1_t, s2_t)):
                            nc.tensor.matmul(
                                out=pd[:, :c1 - c0],
                                lhsT=w_sb[:, wof + ki * F:wof + (ki + 1) * F],
                                rhs=rhs_t[:, c0:c1],
                                start=(ki == 0),
                                stop=(ki == 2),
                            )
                        nc.scalar.activation(
                            out=h_t[:, c0:c1], in_=pd[:, :c1 - c0],
                            func=mybir.ActivationFunctionType.Relu,
                            bias=bias_sb[:, layer:layer + 1],
                        )

            # ---------- pipeline ----------
            obs_t = res.tile([1, 1], dt.int32)
            nc.gpsimd.tensor_copy(out=obs_t[:], in_=src_sb[0:1, 0:1])

            # x: node-major into s_nm, transpose to feature-major, allgather table
            x_nm = bx_bf[0:XLEN].rearrange("(j p f) -> p j f", p=P, f=F)
            nc.sync.dma_start(out=s_nm[:].rearrange("p (j f) -> p j f", f=F),
                              in_=x_nm)
            # collectives may not read IO tensors: bounce x through internal DRAM
            x_nf = bx_bf[0:XLEN].rearrange("(n f) -> n f", f=F)
            nc.sync.dma_start(out=bounce_x[:], in_=x_nf)
            allgather(bounce_x[:], ag_x, "x")
            snm_to_fm(fm["tx0"], "tx0")

            # Layer 1
            prop(ag_x[:], "l1a", probe=True)          # s_nm = Tx1 own (node-major)
            table_write_and_ag(0)                     # ag[0] = Tx1 full
            snm_to_fm(fm["t1"], "t1")
            prop(ag[0][:], "l1b", probe=True)         # s_nm = L@Tx1 own
            snm_to_fm(fm["s2"], "s2")
            dense(0, fm["tx0"], fm["t1"], fm["s2"], fm["h"])
            fm_to_snm(fm["h"], "h1")
            table_write_and_ag(1)                     # ag[1] = h1 full

            # Layer 2
            prop(ag[1][:], "l2a", probe=True)
            table_write_and_ag(2)                     # ag[2] = Tx1' full
            snm_to_fm(fm["t1"], "t1b")
            prop(ag[2][:], "l2b", probe=True)
            snm_to_fm(fm["s2"], "s2b")
            dense(1, fm["h"], fm["t1"], fm["s2"], fm["tx0"])   # h2 -> fm["tx0"]

            # final linear [2 x SHARD]
            with nc.named_scope("final"):
                nchunks = (SHARD + NCHUNK - 1) // NCHUNK
                for ci in range(nchunks):
                    c0 = ci * NCHUNK
                    c1 = min(SHARD, c0 + NCHUNK)
                    pf = pdense.tile([2, NCHUNK], dt.float32, space="PSUM", tag="pd")
                    nc.tensor.matmul(out=pf[:, :c1 - c0], lhsT=wlin_sb[:],
                                     rhs=fm["tx0"][:, c0:c1], start=True, stop=True)
                    ot = spool.tile([2, NCHUNK], dt.float32, tag="ot")
                    nc.scalar.activation(
                        out=ot[:, :c1 - c0], in_=pf[:, :c1 - c0],
                        func=mybir.ActivationFunctionType.Identity,
                        bias=blin_sb[:],
                    )
                    nc.sync.dma_start(out=fo[:, c0:c1], in_=ot[:, :c1 - c0])
                if DBG_NO_AG:
                    for rr in range(C):
                        nc.sync.dma_start(out=ag_o[2 * rr:2 * rr + 2, :], in_=fo[:])
                else:
                    nc.gpsimd.collective_compute(
                        "AllGather",
                        mybir.AluOpType.bypass,
                        replica_groups=rg,
                        ins=[fo[:]],
                        outs=[ag_o.opt()],
                    )
                nc.sync.dma_start(out=out_d[:], in_=ag_o[:])

    nc.compile()
    return nc


# --------------------------------------------------------------------------
# cached PJRT runner (jit built once; per-call = transfer + execute)
# --------------------------------------------------------------------------
def _get_runtime():
    global _RT
    if _RT is not None:
        return _RT
    import jax
    from jax.sharding import Mesh, PartitionSpec, NamedSharding
    from jax.experimental.shard_map import shard_map
    from concourse.bass2jax import (
        _bass_exec_p, install_neuronx_cc_hook, partition_id_tensor)

    # persistent XLA executable cache: the emitted HLO (incl. the embedded
    # BIR) is byte-deterministic, so later processes skip the multi-minute
    # neuronxcc compile entirely
    try:
        jax.config.update("jax_compilation_cache_dir", "/tmp/jax_bass_cache")
    except Exception:
        pass

    nc = _build_kernel()
    install_neuronx_cc_hook()

    partition_name = nc.partition_id_tensor.name if nc.partition_id_tensor else None
    in_names, out_names, out_avals = [], [], []
    for alloc in nc.m.functions[0].allocations:
        if not isinstance(alloc, mybir.MemoryLocationSet):
            continue
        name = alloc.memorylocations[0].name
        if alloc.kind == "ExternalInput":
            if name != partition_name:
                in_names.append(name)
        elif alloc.kind == "ExternalOutput":
            out_names.append(name)
            shape = tuple(alloc.tensor_shape)
            dtype = mybir.dt.np(alloc.dtype)
            out_avals.append(jax.core.ShapedArray(shape, dtype))
    n_params = len(in_names)
    # NOTE: no donated zero-output operands. The axon/NKI lowering passes
    # lowering_input_output_aliases=() so outputs are allocated fresh via
    # nl.ndarray(shared_hbm) regardless, and this kernel writes every output
    # element, so uninitialized output buffers are fine. Skipping the zeros
    # saves one host->device transfer per call.
    if partition_name is not None:
        in_names.append(partition_name)

    def _body(*args):
        operands = list(args)
        if partition_name is not None:
            operands.append(partition_id_tensor())
        outs = _bass_exec_p.bind(
            *operands, out_avals=tuple(out_avals), in_names=tuple(in_names),
            out_names=tuple(out_names), lowering_input_output_aliases=(),
            sim_require_finite=True, sim_require_nnan=True, nc=nc)
        return tuple(outs)

    devices = jax.devices()[:C]
    mesh = Mesh(np.asarray(devices), ("core",))
    in_specs = (PartitionSpec("core"),) * n_params
    out_specs = (PartitionSpec("core"),) * len(out_names)
    sharded = jax.jit(
        shard_map(_body, mesh=mesh, in_specs=in_specs, out_specs=out_specs,
                  check_rep=False),
        keep_unused=True)
    sh = NamedSharding(mesh, PartitionSpec("core"))

    _RT = dict(nc=nc, sharded=sharded, sh=sh, n_params=n_params,
               in_names=in_names, out_names=out_names, jax=jax)
    return _RT


# --------------------------------------------------------------------------
# entry point
# --------------------------------------------------------------------------
def kernel(x, edge_index, edge_weight, W1, b1, W2, b2, Wlin, blin,
           _trace=False, _tmpdir=None):
    import time as _time
    _t0 = _time.perf_counter()
    _dbg = bool(int(os.environ.get("KDBG_T", "0")))
    def _tick(tag):
        if _dbg:
            print(f"  [kt] {tag}: {_time.perf_counter()-_t0:.3f}s", flush=True)
    rt = _get_runtime()
    jax = rt["jax"]
    _tick("runtime")

    x = np.asarray(x, np.float32)
    src = np.asarray(edge_index[0]).astype(np.int32, copy=False)
    dst = np.asarray(edge_index[1]).astype(np.int32, copy=False)
    w = np.asarray(edge_weight, np.float32)

    norm_w, new_id, perm_aux = _norm_and_perm(src, dst, w)
    _tick("norm_perm")

    # x blob ships first: its wire time overlaps the edge packing below
    xbits = x.astype(BF).view(np.uint16)
    bx = np.zeros((NP_PAD, F), np.uint16)
    bx[new_id] = xbits
    _tick("xblob")
    dev_x = jax.device_put(bx.reshape(-1), rt["sh"])
    _tick("put_x_issued")

    eb = np.zeros((C, ELEN), np.uint16)
    try:
        _pack_edges(new_id[src], new_id[dst], norm_w, eb)
    except _TileOverflow:
        # pathological degree distribution: rebalance, redo x, repack
        new_id = _repair(perm_aux[0], perm_aux[1], dst, perm_aux[2])
        bx = np.zeros((NP_PAD, F), np.uint16)
        bx[new_id] = xbits
        dev_x = jax.device_put(bx.reshape(-1), rt["sh"])
        _pack_edges(new_id[src], new_id[dst], norm_w, eb)
    _fold_weights(np.asarray(W1, np.float32), np.asarray(b1, np.float32),
                  np.asarray(W2, np.float32), np.asarray(b2, np.float32),
                  np.asarray(Wlin, np.float32), np.asarray(blin, np.float32),
                  eb)
    _tick("eblob_done")
    dev_e = jax.device_put(eb.reshape(-1), rt["sh"])
    _tick("put_e_issued")

    out_arrs = rt["sharded"](dev_x, dev_e)
    # every core holds the AllGathered [2*C, SHARD] output; fetch shard 0
    # only, and arm the async D2H now so the sync roundtrip overlaps the
    # remaining execute time
    shard0 = out_arrs[0].addressable_shards[0].data
    try:
        shard0.copy_to_host_async()
    except Exception:
        pass
    if _dbg:
        jax.block_until_ready(out_arrs)
        _tick("exec_done")
    res = np.asarray(shard0)
    _tick("fetched")
    out_p = res.reshape(C, 2, SHARD).transpose(1, 0, 2).reshape(2, NP_PAD)
    out = out_p.T[new_id].astype(np.float32)         # [N, 2]
    kernel.last_spmd_wall_s = _time.perf_counter() - _t0
    return out


# revision 44
# speedup vs baseline: 1.1060x; 1.1060x over previous
"""Trainium2 Bass kernel for nn_Cheb_35888746725726 (ChebConv K=3 GNN, N=50000,
E=800000, F=H=96, lambda_max=2 -> diag term is 0).

Strategy (8 NeuronCores, node/graph-parallel). The wall-clock of a call is
dominated by host->device transfer over the axon tunnel (~60-77MB/s, one
serialized pipe, ~35ms fixed per device_put, single host CPU), so inputs are
packed into TWO uint16 blobs per core (~1.72MB/core total, 13.76MB global)
and everything derivable is rebuilt on device:
 - x ships once (bf16, sharded node-major, put FIRST so its wire time
   overlaps the edge packing); an AllGather rebuilds the full node table.
 - The per-edge-tile weighted one-hot scatter matrices (128 edges x 64 dst
   slots) are built on device from compact slot/weight data via DVE
   is_equal+mult against an iota row; slots ship as uint8 pairs packed in
   uint16 and are unpacked with bitwise ops.
 - Dense weights/biases ship sharded 1/8 per core and are AllGathered.
 - Feature-major x comes from on-device PE transposes.
 - Gather indices ship as uint16 and are cast-copied to int32 on device.
 - The output is AllGathered on device so the host fetches a single shard,
   with copy_to_host_async armed at dispatch to hide the D2H sync.
Device pipeline per prop: indirect-DMA gather of source rows (bf16) from the
HBM node table -> scatter via one-hot matmuls accumulating in PSUM. Chebyshev
recurrence folded into host-modified dense weights:
  out = Tx0 @ (W0-W2) + Tx1 @ W1 + (L@Tx1) @ (2*W2), so Tx2 is never formed.
AllGather (8 cores) rebuilds the full node table between dependent props.
The PJRT executable is compiled once and cached; per-call work is vectorized
numpy preprocessing overlapped with the async device_put of the blobs.
"""
import numpy as np
import ml_dtypes

import concourse.bass as bass
import concourse.bacc as bacc
import concourse.mybir as mybir
import concourse.tile as tile
from concourse.masks import make_identity

# ---- problem constants (hardcoded per the harness contract) ----
N = 50000
E = 800000
F = 96
K = 3
C = 8                    # cores
NP_PAD = 50176           # 8 * 6272
SHARD = NP_PAD // C      # 6272
NTW = 64                 # node-tile width
NT = SHARD // NTW        # 98 node tiles / core
TE = 8                   # edge tiles (of 128 edges) per node tile
P = 128
NCHUNK = 512             # dense matmul node-chunk
N_TILES = NP_PAD // NTW  # 784 node tiles globally

BF = ml_dtypes.bfloat16

# two input blobs per core (uint16 units): the x blob ships first so its
# wire time overlaps the edge packing; the edge blob follows.
XLEN = SHARD * F                     # 602112  : x_own bf16 [SHARD, F]
E_SLOTP = 0                          # uint16 [P, NT*TE/2]: uint8 slot PAIRS
E_W = E_SLOTP + P * NT * TE // 2     # bf16 [P, NT*TE] edge weight
E_SRC = E_W + P * NT * TE            # uint16 [P, NT*TE] src node id
E_W8 = E_SRC + P * NT * TE           # this core's 1/8 slice of the weights
# weights are sharded across cores and AllGathered on device. Full layout
# (u16 units, within the gathered buffer):
W_WM = 0                             # bf16 [6F, F] folded dense weights
W_WLIN = W_WM + 6 * F * F            # bf16 [F, 2]
W_B = W_WLIN + F * 2                 # f32 [F, 2] biases (b1, b2 cols)
W_BLIN = W_B + 2 * F * 2             # f32 [2]
WLEN = W_BLIN + 4                    # 55876
WSH = 6992                           # per-core slice (8*WSH = 55936 >= WLEN)
ELEN = E_W8 + WSH

import os
DBG_NO_AG = bool(int(os.environ.get("KDBG_NO_AG", "0")))

_RT = None               # cached runtime: nc + jitted executable + metadata


# --------------------------------------------------------------------------
# host-side preprocessing (fully vectorized)
# --------------------------------------------------------------------------
def _norm_and_perm(src, dst, w):
    """Chebyshev edge norm + degree-balanced node permutation (snake)."""
    deg = np.bincount(src, weights=w.astype(np.float64), minlength=N)
    deg = deg.astype(np.float32)
    dis = np.where(deg > 0, 1.0 / np.sqrt(np.maximum(deg, 1e-30)), 0.0)
    dis = dis.astype(np.float32)
    # positive weights: the Laplacian's minus sign is folded into the W[1]
    # dense blocks (_fold_weights), saving a full negation pass here
    norm_w = (dis[src] * w * dis[dst]).astype(np.float32)

    # snake assignment of degree-sorted nodes -> balanced per-tile edge load
    indeg = np.bincount(dst, minlength=N).astype(np.int32)
    order = np.argsort(-indeg, kind="stable")
    r = np.arange(N, dtype=np.int32)
    blk = r // N_TILES
    pos = r % N_TILES
    t_r = np.where(blk % 2 == 0, pos, N_TILES - 1 - pos)
    tile_assign = np.empty(N, np.int32)
    tile_assign[order] = t_r
    slot_assign = np.empty(N, np.int32)
    slot_assign[order] = blk

    new_id = (tile_assign * NTW + slot_assign).astype(np.int32)
    return norm_w, new_id, (tile_assign, slot_assign, indeg)


class _TileOverflow(Exception):
    pass


def _repair(tile_assign, slot_assign, dst, indeg):
    """Slow path: rebalance tiles whose edge load exceeds TE*P (never taken
    for the reference input distribution; pack_edges detects the overflow
    for free from its sort)."""
    cap = TE * P
    tl = np.bincount(tile_assign[dst], minlength=N_TILES)
    for _ in range(2000):
        if tl.max() <= cap:
            break
        t_over = int(tl.argmax())
        t_under = int(tl.argmin())
        no = np.where(tile_assign == t_over)[0]
        nu = np.where(tile_assign == t_under)[0]
        a = no[np.argmax(indeg[no])]
        b = nu[np.argmin(indeg[nu])]
        tile_assign[a], tile_assign[b] = t_under, t_over
        slot_assign[a], slot_assign[b] = slot_assign[b], slot_assign[a]
        d = indeg[a] - indeg[b]
        tl[t_over] -= d
        tl[t_under] += d
    assert tl.max() <= cap, f"tile overflow after repair: {tl.max()}"
    return (tile_assign * NTW + slot_assign).astype(np.int32)


def _pack_edges(src_n, dst_n, norm_w, blob):
    """Fill per-core slot/weight/src [P, NT*TE] regions of the blob.

    One global radix sort by destination tile; edges of a tile fill its
    TE*P scatter slots in sorted order (the slot one-hot makes intra-tile
    order irrelevant). All scatters run on native uint8/uint16/int32 for
    speed on the single host CPU. Slots ship as uint8 pairs packed into
    uint16 (unpacked on device with bitwise ops).
    """
    slot8 = (dst_n & 63).astype(np.uint8)
    es16 = src_n.astype(np.uint16)
    tof16 = (dst_n >> 6).astype(np.int16)          # global tile id < 784
    o = np.argsort(tof16, kind="stable")           # radix sort
    tof = tof16[o].astype(np.int32)
    slot_s = slot8[o]
    es_s = es16[o]
    wbf = norm_w[o].astype(BF).view(np.uint16)
    starts = np.searchsorted(tof, np.arange(N_TILES, dtype=np.int32))
    if np.diff(np.append(starts, E)).max() > TE * P:
        raise _TileOverflow
    rank = np.arange(E, dtype=np.int32) - starts[tof].astype(np.int32)
    tid = np.arange(N_TILES, dtype=np.int32)
    base784 = ((tid // NT) * P) * (NT * TE) + (tid % NT) * TE
    lin = base784[tof] + (rank & (P - 1)) * (NT * TE) + (rank >> 7)

    npc = P * NT * TE
    sl = np.zeros(C * npc, np.uint8)
    sl[lin] = slot_s
    st = np.zeros((2, C * npc), np.uint16)
    st[0, lin] = wbf
    st[1, lin] = es_s

    blob[:, E_SLOTP:E_W] = sl.view(np.uint16).reshape(C, npc // 2)
    blob[:, E_W:E_SRC] = st[0].reshape(C, npc)
    blob[:, E_SRC:E_W8] = st[1].reshape(C, npc)


def _fold_weights(W1, b1, W2, b2, Wlin, blin, blob):
    # device props apply +|L| (positive edge weights), so the on-device
    # t1 = -Tx1 while s2 = |L|@|L|@x = L@Tx1 is unchanged: negate the W[1]
    # blocks to compensate
    wm = np.concatenate([
        W1[0] - W1[2], -W1[1], 2.0 * W1[2],
        W2[0] - W2[2], -W2[1], 2.0 * W2[2],
    ], axis=0).astype(BF)                                     # [6F, F]
    wall = np.zeros(C * WSH, np.uint16)
    wall[W_WM:W_WLIN] = wm.reshape(-1).view(np.uint16)
    wall[W_WLIN:W_B] = Wlin.astype(BF).reshape(-1).view(np.uint16)
    biases = np.stack([b1, b2], axis=1).astype(np.float32)    # [F, 2]
    wall[W_B:W_BLIN] = biases.reshape(-1).view(np.uint16)
    wall[W_BLIN:WLEN] = blin.astype(np.float32).reshape(-1).view(np.uint16)
    blob[:, E_W8:ELEN] = wall.reshape(C, WSH)


# --------------------------------------------------------------------------
# bass kernel builder
# --------------------------------------------------------------------------
def _build_kernel():
    dt = mybir.dt
    nc = bacc.Bacc("TRN2", target_bir_lowering=False, debug=False, num_devices=C)

    bx_d = nc.dram_tensor("bx", [XLEN], dt.uint16, kind="ExternalInput")
    be_d = nc.dram_tensor("be", [ELEN], dt.uint16, kind="ExternalInput")
    # full output is AllGathered on device so the host fetches ONE shard
    out_d = nc.dram_tensor("out", [2 * C, SHARD], dt.float32, kind="ExternalOutput")

    bx_bf = bx_d.bitcast(dt.bfloat16)
    be_bf = be_d.bitcast(dt.bfloat16)

    rg = [list(range(C))]

    with tile.TileContext(nc) as tc:
        with (
            tc.tile_pool(name="res", bufs=1) as res,          # resident sbuf
            tc.tile_pool(name="mpool", bufs=4) as mpool,      # gather dests
            tc.tile_pool(name="spool", bufs=2) as spool,      # small evac tiles
            tc.tile_pool(name="pscat", bufs=4, space="PSUM") as pscat,
            tc.tile_pool(name="ptr", bufs=2, space="PSUM") as ptr,
            tc.tile_pool(name="pdense", bufs=2, space="PSUM") as pdense,
            tc.tile_pool(name="dram", bufs=1, space="DRAM") as dram,
        ):
            # ---------- resident loads (from the packed blobs) ----------
            # weights: each core ships 1/8; AllGather rebuilds the full set
            wb = dram.tile([WSH], dt.uint16, name="wbounce")
            wfull = dram.tile([C * WSH], dt.uint16,
                              addr_space=("Local" if DBG_NO_AG else "Shared"),
                              name="wfull")
            nc.sync.dma_start(out=wb[:], in_=be_d[E_W8:ELEN])
            if DBG_NO_AG:
                for r_ in range(C):
                    nc.sync.dma_start(out=wfull[r_ * WSH:(r_ + 1) * WSH], in_=wb[:])
            else:
                nc.gpsimd.collective_compute(
                    "AllGather", mybir.AluOpType.bypass, replica_groups=rg,
                    ins=[wb[:]], outs=[wfull.opt()])

            # slot pairs -> f32 slot_sb via bitwise unpack (tensor_scalar
            # is_equal requires float32 scalar operands)
            slotp_sb = res.tile([P, NT * TE // 2], dt.uint16)
            nc.sync.dma_start(out=slotp_sb[:],
                              in_=be_d[E_SLOTP:E_W].rearrange("(p c) -> p c", p=P))
            lo_u = res.tile([P, NT * TE // 2], dt.uint16)
            hi_u = res.tile([P, NT * TE // 2], dt.uint16)
            nc.vector.tensor_scalar(out=lo_u[:], in0=slotp_sb[:], scalar1=0x00FF,
                                    scalar2=None, op0=mybir.AluOpType.bitwise_and)
            nc.vector.tensor_scalar(out=hi_u[:], in0=slotp_sb[:], scalar1=0xFF00,
                                    scalar2=None, op0=mybir.AluOpType.bitwise_and)
            slot_sb = res.tile([P, NT * TE], dt.float32)
            slot_ev = slot_sb[:].rearrange("p (c two) -> p c two", two=2)
            nc.vector.tensor_copy(
                out=slot_ev[:, :, 0:1].rearrange("p c o -> p (c o)"), in_=lo_u[:])
            hif = res.tile([P, NT * TE // 2], dt.float32)
            nc.vector.tensor_copy(out=hif[:], in_=hi_u[:])
            nc.vector.tensor_scalar(
                out=slot_ev[:, :, 1:2].rearrange("p c o -> p (c o)"),
                in0=hif[:], scalar1=1.0 / 256.0, scalar2=None,
                op0=mybir.AluOpType.mult)

            we_sb = res.tile([P, NT * TE], dt.float32)
            nc.gpsimd.dma_start(out=we_sb[:],
                                in_=be_bf[E_W:E_SRC].rearrange("(p c) -> p c", p=P))
            srcu_sb = res.tile([P, NT * TE], dt.uint16)
            nc.sync.dma_start(out=srcu_sb[:],
                              in_=be_d[E_SRC:E_W8].rearrange("(p c) -> p c", p=P))
            src_sb = res.tile([P, NT * TE], dt.int32)
            nc.vector.tensor_copy(out=src_sb[:], in_=srcu_sb[:])

            w_sb = res.tile([F, 6 * F], dt.bfloat16)   # 6 lhsT mats side by side
            for i in range(6):
                nc.sync.dma_start(
                    out=w_sb[:, i * F:(i + 1) * F],
                    in_=wfull[W_WM + i * F * F:W_WM + (i + 1) * F * F]
                        .bitcast(dt.bfloat16).rearrange("(a b) -> a b", b=F))
            wlin_sb = res.tile([F, 2], dt.bfloat16)
            nc.sync.dma_start(out=wlin_sb[:],
                              in_=wfull[W_WLIN:W_B].bitcast(dt.bfloat16)
                                  .rearrange("(f b) -> f b", b=2))
            bias_sb = res.tile([F, 2], dt.float32)
            nc.sync.dma_start(out=bias_sb[:],
                              in_=wfull[W_B:W_BLIN].bitcast(dt.float32)
                                  .rearrange("(f b) -> f b", b=2))
            blin_sb = res.tile([2, 1], dt.float32)
            nc.sync.dma_start(out=blin_sb[:],
                              in_=wfull[W_BLIN:WLEN].bitcast(dt.float32)
                                  .rearrange("(p o) -> p o", o=1))
            ident = res.tile([P, P], dt.bfloat16)
            make_identity(nc, ident[:])

            # one-hot scatter matrices built on device: oh[p, e*64+s] =
            # (s == slot[p,e]) * w[p,e]
            iota_i = res.tile([P, NTW], dt.int32)
            nc.gpsimd.iota(iota_i[:], pattern=[[1, NTW]], base=0,
                           channel_multiplier=0)
            iota_bf = res.tile([P, NTW], dt.bfloat16)
            nc.vector.tensor_copy(out=iota_bf[:], in_=iota_i[:])
            oh_sb = res.tile([P, NT * TE * NTW], dt.bfloat16)
            for e in range(NT * TE):
                nc.vector.tensor_scalar(
                    out=oh_sb[:, e * NTW:(e + 1) * NTW],
                    in0=iota_bf[:],
                    scalar1=slot_sb[:, e:e + 1],
                    scalar2=we_sb[:, e:e + 1],
                    op0=mybir.AluOpType.is_equal,
                    op1=mybir.AluOpType.mult,
                )

            # feature-major activation buffers (bf16)
            fm = {
                "tx0": res.tile([F, SHARD], dt.bfloat16, name="fm_tx0"),
                "t1": res.tile([F, SHARD], dt.bfloat16, name="fm_t1"),
                "s2": res.tile([F, SHARD], dt.bfloat16, name="fm_s2"),
                "h": res.tile([F, SHARD], dt.bfloat16, name="fm_h"),
            }

            # node-major staging for table writes / transposes
            s_nm = res.tile([P, (NT // 2) * F], dt.bfloat16)

            # internal DRAM
            bounce = [dram.tile([SHARD, F], dt.bfloat16, name=f"bounce{i}") for i in range(3)]
            bounce_x = dram.tile([SHARD, F], dt.bfloat16, name="bounce_x")
            fo = dram.tile([2, SHARD], dt.float32, name="fo")
            ag_o = dram.tile([2 * C, SHARD], dt.float32,
                             addr_space=("Local" if DBG_NO_AG else "Shared"),
                             name="ag_o")
            addr_space = "Local" if DBG_NO_AG else "Shared"
            ag = [dram.tile([NP_PAD, F], dt.bfloat16,
                            addr_space=addr_space, name=f"ag{i}")
                  for i in range(3)]
            ag_x = dram.tile([NP_PAD, F], dt.bfloat16,
                             addr_space=addr_space, name="ag_x")

            # ---------- helpers ----------
            def prop(table_ap, tag, probe=False):
                """one propagation: gather+scatter; results land in s_nm (node-major)."""
                with nc.named_scope(f"prop_{tag}"):
                    pr = None
                    if probe:
                        # tiny gpsimd-issued DMA touching the table: executes the
                        # collective-completion wait so the 1-wait-limited
                        # dynamic gathers below don't need it
                        pr = spool.tile([1, 2], dt.bfloat16, tag="pr")
                        nc.gpsimd.dma_start(out=pr[:], in_=table_ap.tensor[0:1, 0:2])
                    for nt in range(NT):
                        m_t = mpool.tile([P, TE * F], dt.bfloat16, tag="m")
                        # absorber: one strided gpsimd write touching each edge
                        # tile's corner carries the slot's WAR/WAW waits (and the
                        # table-probe dep for the first tile) so each 1-wait-
                        # limited dynamic gather below needs at most one wait.
                        # HW note: indirect DMA honors only ONE offset column
                        # per call, hence one gather per 128-edge tile.
                        if pr is not None and nt == 0:
                            nc.gpsimd.tensor_copy(out=m_t[0:1, 0:1], in_=pr[0:1, 0:1])
                        corner = m_t[:].rearrange("p (t f) -> p t f", f=F)[0:1, :, 0:1]
                        nc.gpsimd.memset(corner, 0)
                        for t in range(TE):
                            nc.gpsimd.indirect_dma_start(
                                out=m_t[:, t * F:(t + 1) * F],
                                out_offset=None,
                                in_=table_ap,
                                in_offset=bass.IndirectOffsetOnAxis(
                                    ap=src_sb[:, nt * TE + t:nt * TE + t + 1], axis=0),
                            )
                        ps = pscat.tile([NTW, F], dt.float32, space="PSUM", tag="ps")
                        for t in range(TE):
                            nc.tensor.matmul(
                                out=ps[:],
                                lhsT=oh_sb[:, (nt * TE + t) * NTW:(nt * TE + t + 1) * NTW],
                                rhs=m_t[:, t * F:(t + 1) * F],
                                start=(t == 0),
                                stop=(t == TE - 1),
                            )
                        j, b = nt // 2, nt % 2
                        nc.vector.tensor_copy(
                            out=s_nm[b * NTW:(b + 1) * NTW, j * F:(j + 1) * F],
                            in_=ps[:])

            def allgather(in_ap, out_tile, tag):
                with nc.named_scope(f"ag_{tag}"):
                    if DBG_NO_AG:
                        for r in range(C):
                            nc.sync.dma_start(
                                out=out_tile[r * SHARD:(r + 1) * SHARD, :],
                                in_=in_ap)
                    else:
                        nc.gpsimd.collective_compute(
                            "AllGather",
                            mybir.AluOpType.bypass,
                            replica_groups=rg,
                            ins=[in_ap],
                            outs=[out_tile.opt()],
                        )

            def table_write_and_ag(idx):
                """write s_nm -> bounce[idx] (node-major [SHARD, F]) and allgather."""
                bo = bounce[idx]
                view = bo[:].rearrange("(j p) f -> p j f", p=P)
                nc.sync.dma_start(out=view, in_=s_nm[:].rearrange("p (j f) -> p j f", f=F))
                allgather(bo[:], ag[idx], str(idx))

            def snm_to_fm(dest, tag):
                """transpose node-major s_nm into feature-major dest tile."""
                with nc.named_scope(f"fm_{tag}"):
                    for j in range(NT // 2):
                        pt = ptr.tile([F, P], dt.bfloat16, space="PSUM", tag="pt")
                        nc.tensor.transpose(out=pt[:], in_=s_nm[:, j * F:(j + 1) * F],
                                            identity=ident[:])
                        nc.vector.tensor_copy(out=dest[:, j * P:(j + 1) * P], in_=pt[:])

            def fm_to_snm(src_t, tag):
                """transpose feature-major tile back into s_nm node-major staging."""
                with nc.named_scope(f"nm_{tag}"):
                    for j in range(NT // 2):
                        pt = ptr.tile([P, F], dt.bfloat16, space="PSUM", tag="pt")
                        nc.tensor.transpose(out=pt[:], in_=src_t[:, j * P:(j + 1) * P],
                                            identity=ident[:F, :F])
                        nc.vector.tensor_copy(out=s_nm[:, j * F:(j + 1) * F], in_=pt[:])

            def dense(layer, tx0_t, t1_t, s2_t, h_t):
                """h = relu(tx0@W0' + t1@W1 + s2@W2') feature-major, bf16 out."""
                with nc.named_scope(f"dense_{layer}"):
                    wof = layer * 3 * F
                    nchunks = (SHARD + NCHUNK - 1) // NCHUNK
                    for ci in range(nchunks):
                        c0 = ci * NCHUNK
                        c1 = min(SHARD, c0 + NCHUNK)
                        pd = pdense.tile([F, NCHUNK], dt.float32, space="PSUM", tag="pd")
                        for ki, rhs_t in enumerate((tx0_t, t1_t, s2_t)):
                            nc.tensor.matmul(
                                out=pd[:, :c1 - c0],
                                lhsT=w_sb[:, wof + ki * F:wof + (ki + 1) * F],
                                rhs=rhs_t[:, c0:c1],
                                start=(ki == 0),
                                stop=(ki == 2),
                            )
                        nc.scalar.activation(
                            out=h_t[:, c0:c1], in_=pd[:, :c1 - c0],
                            func=mybir.ActivationFunctionType.Relu,
                            bias=bias_sb[:, layer:layer + 1],
                        )

            # ---------- pipeline ----------
            obs_t = res.tile([1, 1], dt.int32)
            nc.gpsimd.tensor_copy(out=obs_t[:], in_=src_sb[0:1, 0:1])

            # x: node-major into s_nm, transpose to feature-major, allgather table
            x_nm = bx_bf[0:XLEN].rearrange("(j p f) -> p j f", p=P, f=F)
            nc.sync.dma_start(out=s_nm[:].rearrange("p (j f) -> p j f", f=F),
                              in_=x_nm)
            # collectives may not read IO tensors: bounce x through internal DRAM
            x_nf = bx_bf[0:XLEN].rearrange("(n f) -> n f", f=F)
            nc.sync.dma_start(out=bounce_x[:], in_=x_nf)
            allgather(bounce_x[:], ag_x, "x")
            snm_to_fm(fm["tx0"], "tx0")

            # Layer 1
            prop(ag_x[:], "l1a", probe=True)          # s_nm = Tx1 own (node-major)
            table_write_and_ag(0)                     # ag[0] = Tx1 full
            snm_to_fm(fm["t1"], "t1")
            prop(ag[0][:], "l1b", probe=True)         # s_nm = L@Tx1 own
            snm_to_fm(fm["s2"], "s2")
            dense(0, fm["tx0"], fm["t1"], fm["s2"], fm["h"])
            fm_to_snm(fm["h"], "h1")
            table_write_and_ag(1)                     # ag[1] = h1 full

            # Layer 2
            prop(ag[1][:], "l2a", probe=True)
            table_write_and_ag(2)                     # ag[2] = Tx1' full
            snm_to_fm(fm["t1"], "t1b")
            prop(ag[2][:], "l2b", probe=True)
            snm_to_fm(fm["s2"], "s2b")
            dense(1, fm["h"], fm["t1"], fm["s2"], fm["tx0"])   # h2 -> fm["tx0"]

            # final linear [2 x SHARD]
            with nc.named_scope("final"):
                nchunks = (SHARD + NCHUNK - 1) // NCHUNK
                for ci in range(nchunks):
                    c0 = ci * NCHUNK
                    c1 = min(SHARD, c0 + NCHUNK)
                    pf = pdense.tile([2, NCHUNK], dt.float32, space="PSUM", tag="pd")
                    nc.tensor.matmul(out=pf[:, :c1 - c0], lhsT=wlin_sb[:],
                                     rhs=fm["tx0"][:, c0:c1], start=True, stop=True)
                    ot = spool.tile([2, NCHUNK], dt.float32, tag="ot")
                    nc.scalar.activation(
                        out=ot[:, :c1 - c0], in_=pf[:, :c1 - c0],
                        func=mybir.ActivationFunctionType.Identity,
                        bias=blin_sb[:],
                    )
                    nc.sync.dma_start(out=fo[:, c0:c1], in_=ot[:, :c1 - c0])
                if DBG_NO_AG:
                    for rr in range(C):
                        nc.sync.dma_start(out=ag_o[2 * rr:2 * rr + 2, :], in_=fo[:])
                else:
                    nc.gpsimd.collective_compute(
                        "AllGather",
                        mybir.AluOpType.bypass,
                        replica_groups=rg,
                        ins=[fo[:]],
                        outs=[ag_o.opt()],
                    )
                nc.sync.dma_start(out=out_d[:], in_=ag_o[:])

    nc.compile()
    return nc


# --------------------------------------------------------------------------
# cached PJRT runner (jit built once; per-call = transfer + execute)
# --------------------------------------------------------------------------
def _get_runtime():
    global _RT
    if _RT is not None:
        return _RT
    import jax
    from jax.sharding import Mesh, PartitionSpec, NamedSharding
    from jax.experimental.shard_map import shard_map
    from concourse.bass2jax import (
        _bass_exec_p, install_neuronx_cc_hook, partition_id_tensor)

    # persistent XLA executable cache: the emitted HLO (incl. the embedded
    # BIR) is byte-deterministic, so later processes skip the multi-minute
    # neuronxcc compile entirely
    try:
        jax.config.update("jax_compilation_cache_dir", "/tmp/jax_bass_cache")
    except Exception:
        pass

    nc = _build_kernel()
    install_neuronx_cc_hook()

    partition_name = nc.partition_id_tensor.name if nc.partition_id_tensor else None
    in_names, out_names, out_avals = [], [], []
    for alloc in nc.m.functions[0].allocations:
        if not isinstance(alloc, mybir.MemoryLocationSet):
            continue
        name = alloc.memorylocations[0].name
        if alloc.kind == "ExternalInput":
            if name != partition_name:
                in_names.append(name)
        elif alloc.kind == "ExternalOutput":
            out_names.append(name)
            shape = tuple(alloc.tensor_shape)
            dtype = mybir.dt.np(alloc.dtype)
            out_avals.append(jax.core.ShapedArray(shape, dtype))
    n_params = len(in_names)
    # NOTE: no donated zero-output operands. The axon/NKI lowering passes
    # lowering_input_output_aliases=() so outputs are allocated fresh via
    # nl.ndarray(shared_hbm) regardless, and this kernel writes every output
    # element, so uninitialized output buffers are fine. Skipping the zeros
    # saves one host->device transfer per call.
    if partition_name is not None:
        in_names.append(partition_name)

    def _body(*args):
        operands = list(args)
        if partition_name is not None:
            operands.append(partition_id_tensor())
        outs = _bass_exec_p.bind(
            *operands, out_avals=tuple(out_avals), in_names=tuple(in_names),
            out_names=tuple(out_names), lowering_input_output_aliases=(),
            sim_require_finite=True, sim_require_nnan=True, nc=nc)
        return tuple(outs)

    devices = jax.devices()[:C]
    mesh = Mesh(np.asarray(devices), ("core",))
    in_specs = (PartitionSpec("core"),) * n_params
    out_specs = (PartitionSpec("core"),) * len(out_names)
    sharded = jax.jit(
        shard_map(_body, mesh=mesh, in_specs=in_specs, out_specs=out_specs,
                  check_rep=False),
        keep_unused=True)
    sh = NamedSharding(mesh, PartitionSpec("core"))

    _RT = dict(nc=nc, sharded=sharded, sh=sh, n_params=n_params,
               in_names=in_names, out_names=out_names, jax=jax)
    return _RT


# --------------------------------------------------------------------------
# entry point
# --------------------------------------------------------------------------
def kernel(x, edge_index, edge_weight, W1, b1, W2, b2, Wlin, blin,
           _trace=False, _tmpdir=None):
    import time as _time
    _t0 = _time.perf_counter()
    _dbg = bool(int(os.environ.get("KDBG_T", "0")))
    def _tick(tag):
        if _dbg:
            print(f"  [kt] {tag}: {_time.perf_counter()-_t0:.3f}s", flush=True)
    rt = _get_runtime()
    jax = rt["jax"]
    _tick("runtime")

    x = np.asarray(x, np.float32)
    src = np.asarray(edge_index[0]).astype(np.int32, copy=False)
    dst = np.asarray(edge_index[1]).astype(np.int32, copy=False)
    w = np.asarray(edge_weight, np.float32)

    norm_w, new_id, perm_aux = _norm_and_perm(src, dst, w)
    _tick("norm_perm")

    # x blob ships first: its wire time overlaps the edge packing below
    xbits = x.astype(BF).view(np.uint16)
    bx = np.zeros((NP_PAD, F), np.uint16)
    bx[new_id] = xbits
    _tick("xblob")
    dev_x = jax.device_put(bx.reshape(-1), rt["sh"])
    _tick("put_x_issued")

    eb = np.zeros((C, ELEN), np.uint16)
    try:
        _pack_edges(new_id[src], new_id[dst], norm_w, eb)
    except _TileOverflow:
        # pathological degree distribution: rebalance, redo x, repack
        new_id = _repair(perm_aux[0], perm_aux[1], dst, perm_aux[2])
        bx = np.zeros((NP_PAD, F), np.uint16)
        bx[new_id] = xbits
        dev_x = jax.device_put(bx.reshape(-1), rt["sh"])
        _pack_edges(new_id[src], new_id[dst], norm_w, eb)
    _fold_weights(np.asarray(W1, np.float32), np.asarray(b1, np.float32),
                  np.asarray(W2, np.float32), np.asarray(b2, np.float32),
                  np.asarray(Wlin, np.float32), np.asarray(blin, np.float32),
                  eb)
    _tick("eblob_done")
    dev_e = jax.device_put(eb.reshape(-1), rt["sh"])
    _tick("put_e_issued")

    out_arrs = rt["sharded"](dev_x, dev_e)
    # every core holds the AllGathered [2*C, SHARD] output; fetch shard 0
    # only, and arm the async D2H now so the sync roundtrip overlaps the
    # remaining execute time
    shard0 = out_arrs[0].addressable_shards[0].data
    try:
        shard0.copy_to_host_async()
    except Exception:
        pass
    if _dbg:
        jax.block_until_ready(out_arrs)
        _tick("exec_done")
    res = np.asarray(shard0)
    _tick("fetched")
    out_p = res.reshape(C, 2, SHARD).transpose(1, 0, 2).reshape(2, NP_PAD)
    out = out_p.T[new_id].astype(np.float32)         # [N, 2]
    kernel.last_spmd_wall_s = _time.perf_counter() - _t0
    return out
